# revision 25
# baseline (speedup 1.0000x reference)
"""Trainium2 Bass kernel for nn_AttentionLayer (sparse_attention).

Reference computation (B=4, N=2048, C=256, H=8, HD=32):
    qkv = x @ qkv_w.T; q,k,v = split(qkv); heads
    scores = q k^T / sqrt(HD) + adj          [B,H,N,N]
    out    = softmax(scores) @ v             -> merge heads [B,N,C]
    result = (out*0.1 + x) @ out_w.T + out_b
(The pos_proj(adj) value in the reference is dead code; x0 is unused.)

Sharding: 8 cores = (batch b, query-half). Core c handles batch c//2 and
query rows [ (c%2)*1024, (c%2+1)*1024 ).  Each core computes K/V for its
whole batch locally (no collectives).  To keep the SPMD graph identical
across cores, the host rolls the key axis so that the core's own query
rows are always rows 0..1023 of its x input; adj columns are rolled the
same way (softmax is key-permutation invariant).  The host also passes
x / weights pre-transposed (and bf16-cast) and adj pre-transposed so the
device does no layout work.

Per-core kernel math (bf16 matmuls, fp32 psum).  The scores land
transposed ([key, query]) so the attn@v contraction needs no on-device
transpose.  The adj add + exp is split across engines to balance load:
most key tiles:   DVE adds adj (fp32, psum+sbuf->sbuf), ACT exps from SBUF
D_KTS key tiles:  ACT exps the raw qk scores straight from PSUM (slower
                  psum read) and the idle GPSIMD engine multiplies by a
                  precomputed exp(adj) (exp(a+b) = exp(a)exp(b)).
attention out = E @ v_aug with v_aug = [v | 10.0]: row 32 of the transposed
product is 10*sum(E); its reciprocal is the softmax denominator with the
0.1 output scale folded in.  Normalization, the x residual and out_proj
happen on the transposed attention output, which is exactly the layout
out_proj's contraction wants.
"""

import sys

for _p in ("/opt/trn_rl_repo", "/root/.axon_site/_ro/trn_rl_repo"):
    if _p not in sys.path:
        sys.path.insert(0, _p)

import ml_dtypes
import numpy as np

import concourse.mybir as mybir
from concourse import bacc
from concourse.bass import ds, ts
from concourse.tile import TileContext

B, N, C, H = 4, 2048, 256, 8
HD = C // H          # 32
NQ = N // 2          # 1024 query rows per core
SCALE = 1.0 / np.sqrt(HD)
FP32 = mybir.dt.float32
BF16 = mybir.dt.bfloat16
P = 128
BF16NP = ml_dtypes.bfloat16

_CACHED = {}
D_KTS = (2, 5, 8, 11)  # key tiles routed via gpsimd-multiplied factorized path
SP_BUFS = 2
PO_BUFS = 2
PAIR_B = False  # fusing two exps into one ACT op measured slower (pipeline barrier)


def build_kernel(repeat=1):
    nc = bacc.Bacc("TRN2", target_bir_lowering=False)
    xt_ext = nc.declare_dram_parameter("xt", [C, N], BF16, isOutput=False)
    adjt_ext = nc.declare_dram_parameter("adjt", [P, 16, NQ], FP32, isOutput=False)
    wt_ext = nc.declare_dram_parameter("qkv_wt", [C, 3 * C], BF16, isOutput=False)
    owt_ext = nc.declare_dram_parameter("out_wt", [C, C], BF16, isOutput=False)
    outb_ext = nc.declare_dram_parameter("out_b", [P, C], FP32, isOutput=False)
    out_ext = nc.declare_dram_parameter("out", [NQ, C], FP32, isOutput=True)

    with TileContext(nc) as tc:
        with (
            tc.tile_pool(name="const", bufs=1) as constp,
            tc.tile_pool(name="persist", bufs=1) as persist,
            tc.tile_pool(name="stage", bufs=2) as stage,
            tc.tile_pool(name="work", bufs=2) as work,
            tc.tile_pool(name="sp_pool", bufs=SP_BUFS, space="PSUM") as spp,
            tc.tile_pool(name="op", bufs=PO_BUFS, space="PSUM") as op,
        ):
            outb_bc = constp.tile([P, C], FP32)
            nc.sync.dma_start(outb_bc[:], outb_ext[:, :])
            for _ in range(repeat):
                _body(nc, tc, persist, stage, work, spp, op, outb_bc,
                      xt_ext, adjt_ext, wt_ext, owt_ext, out_ext)

    nc.compile()
    return nc


def _body(nc, tc, persist, stage, work, spp, op, outb_bc,
          xt_ext, adjt_ext, wt_ext, owt_ext, out_ext):
    # ---------------- persistent SBUF tensors ----------------
    xT = [persist.tile([P, N], BF16, tag=f"xT{i}", name=f"xT{i}") for i in range(2)]
    wT = [persist.tile([P, 3 * C], BF16, tag=f"wT{i}", name=f"wT{i}") for i in range(2)]
    owT = [persist.tile([P, C], BF16, tag=f"owT{i}", name=f"owT{i}") for i in range(2)]
    kT = [persist.tile([64, N], BF16, tag=f"kT{i}", name=f"kT{i}") for i in range(4)]
    qT = [persist.tile([64, NQ], BF16, tag=f"qT{i}", name=f"qT{i}") for i in range(4)]
    vv = persist.tile([P, 16, H, HD + 1], BF16, tag="vv")
    adjT = persist.tile([P, 16, NQ], FP32, tag="adjT")
    eadjT = persist.tile([P, max(1, len(D_KTS)), NQ], BF16, tag="eadjT")
    attT = [persist.tile([P, NQ], BF16, tag=f"attT{i}", name=f"attT{i}")
            for i in range(2)]

    # ---------------- loads (already transposed/bf16 on host) -------------
    for j in range(2):
        for hseg in range(2):
            nc.sync.dma_start(xT[j][:, ds(hseg * NQ, NQ)],
                              xt_ext[ds(j * P, P), ds(hseg * NQ, NQ)])
        nc.sync.dma_start(wT[j][:], wt_ext[ds(j * P, P), :])
        nc.sync.dma_start(owT[j][:], owt_ext[ds(j * P, P), :])

    # ---------------- QKV projections (bf16) ----------------
    # kT[i][kd_local, key] : k head-dims 64*i+kd_local over all N keys
    for m in range(4):
        for nch in range(4):
            pk = spp.tile([P, 1024], FP32, tag="sp", name="pk")[:, :512]
            for cc in range(2):
                nc.tensor.matmul(pk[:64, :], wT[cc][:, ds(C + m * 64, 64)],
                                 xT[cc][:, ts(nch, 512)],
                                 start=(cc == 0), stop=(cc == 1))
            nc.scalar.copy(kT[m][:, ts(nch, 512)], pk[:64, :])
    # v: [key_tile, head, hd] with ones column scaled by 10 (folds the 0.1)
    nc.vector.memset(vv[:, :, :, HD], 10.0)
    for kt in range(16):
        pv = spp.tile([P, 1024], FP32, tag="sp", name="pv")[:, :512]
        for cc in range(2):
            nc.tensor.matmul(pv[:, :C], xT[cc][:, ts(kt, P)],
                             wT[cc][:, ds(2 * C, C)],
                             start=(cc == 0), stop=(cc == 1))
        nc.scalar.copy(
            vv[:, kt, :, 0:HD],
            pv[:, :C].rearrange("p (h d) -> p h d", h=H))
    # qT (own 1024 rows, scaled by 1/sqrt(HD))
    for m in range(4):
        for nch in range(2):
            pq = spp.tile([P, 1024], FP32, tag="sp", name="pq")[:, :512]
            for cc in range(2):
                nc.tensor.matmul(pq[:64, :], wT[cc][:, ds(m * 64, 64)],
                                 xT[cc][:, ts(nch, 512)],
                                 start=(cc == 0), stop=(cc == 1))
            nc.scalar.mul(qT[m][:, ts(nch, 512)], pq[:64, :], SCALE)

    # ---------------- load adjT (host passed adj transposed) --------------
    for i in range(16):
        nc.gpsimd.dma_start(adjT[:, i, :], adjt_ext[:, i, :])
    for di, kt in enumerate(tuple(D_KTS)):
        nc.scalar.activation(eadjT[:, di, :], adjT[:, kt, :],
                             mybir.ActivationFunctionType.Exp)

    # ---------------- attention: per head, per key tile ----------------
    for h in range(8):
        ht, hr = divmod(h, 2)
        at, ar = divmod(h, 4)
        po = op.tile([HD + 1, NQ], FP32, tag="po", name="po")

        def score_mm(kt):
            sp = spp.tile([P, NQ], FP32, tag="sp", name="sp")
            for c in range(2):
                nc.tensor.matmul(sp[:, ds(c * 512, 512)],
                                 kT[ht][ds(hr * HD, HD), ts(kt, P)],
                                 qT[ht][ds(hr * HD, HD), ds(c * 512, 512)],
                                 start=True, stop=True)
            return sp

        seen = [0]

        def attnv(kt, et_ap):
            seen[0] += 1
            for c in range(2):
                nc.tensor.matmul(po[:, ds(c * 512, 512)],
                                 vv[:, kt, h, :],
                                 et_ap[:, ds(c * 512, 512)],
                                 start=(seen[0] == 1),
                                 stop=(seen[0] == 16),
                                 skip_group_check=True)

        # schedule: B tiles in pairs (one fused ACT exp per pair), D tiles
        # singly on the psum-exp + gpsimd path
        if PAIR_B:
            b_kts = [kt for kt in range(16) if kt not in D_KTS]
            groups = [tuple(b_kts[i:i + 2]) for i in range(0, len(b_kts), 2)]
            for kt in D_KTS:
                groups.insert(kt // 3 if kt // 3 < len(groups) else len(groups),
                              (kt,))
            sched = groups
        else:
            sched = [(kt,) for kt in range(16)]

        for grp in sched:
            if len(grp) == 1 and grp[0] in D_KTS:
                kt = grp[0]
                sp = score_mm(kt)
                e1 = work.tile([P, NQ], BF16, tag="e1", name="e1", bufs=4)
                nc.scalar.activation(e1[:], sp[:],
                                     mybir.ActivationFunctionType.Exp)
                et = work.tile([P, NQ], BF16, tag="et", name="et", bufs=4)
                nc.gpsimd.tensor_tensor(et[:], e1[:],
                                        eadjT[:, D_KTS.index(kt), :],
                                        mybir.AluOpType.mult)
                attnv(kt, et[:])
            elif len(grp) == 2:
                a, b = grp
                sm2 = work.tile([P, 2, NQ], FP32, tag="sm2", name="sm2", bufs=3)
                sp_a = score_mm(a)
                nc.vector.tensor_tensor(sm2[:, 0, :], sp_a[:], adjT[:, a, :],
                                        mybir.AluOpType.add)
                sp_b = score_mm(b)
                nc.vector.tensor_tensor(sm2[:, 1, :], sp_b[:], adjT[:, b, :],
                                        mybir.AluOpType.add)
                et2 = work.tile([P, 2, NQ], BF16, tag="et2", name="et2", bufs=3)
                nc.scalar.activation(et2[:], sm2[:],
                                     mybir.ActivationFunctionType.Exp)
                attnv(a, et2[:, 0, :])
                attnv(b, et2[:, 1, :])
            else:
                kt = grp[0]
                sp = score_mm(kt)
                sm = work.tile([P, NQ], FP32, tag="sm", name="sm", bufs=4)
                nc.vector.tensor_tensor(sm[:], sp[:], adjT[:, kt, :],
                                        mybir.AluOpType.add)
                et = work.tile([P, NQ], BF16, tag="et", name="et", bufs=4)
                nc.scalar.activation(et[:], sm[:],
                                     mybir.ActivationFunctionType.Exp)
                attnv(kt, et[:])
        # normalize rows 0..31 by 1/(10*sum) (0.1 softmax scale folded in)
        rec = work.tile([1, NQ], FP32, tag="rec", name="rec")
        nc.vector.reciprocal(rec[:], po[ds(HD, 1), :])
        bc = work.tile([HD, NQ], FP32, tag="bc", name="bc")
        nc.sync.dma_start(bc[:], rec[:, None, :].to_broadcast((1, HD, NQ)))
        nc.vector.tensor_tensor(attT[at][ds(ar * HD, HD), :], po[0:HD, :], bc[:],
                                mybir.AluOpType.mult)

    # ---------------- residual + out_proj ----------------
    for cc in range(2):
        nc.vector.tensor_tensor(attT[cc][:], attT[cc][:], xT[cc][:, 0:NQ],
                                mybir.AluOpType.add)
    for rt in range(8):
        pf = op.tile([P, 512], FP32, tag="po", name="pf")
        for cc in range(2):
            nc.tensor.matmul(pf[:, :C], attT[cc][:, ts(rt, P)], owT[cc][:],
                             start=(cc == 0), stop=(cc == 1))
        osb = work.tile([P, C], FP32, tag="osb", name="osb")
        nc.vector.tensor_tensor(osb[:], pf[:, :C], outb_bc[:],
                                mybir.AluOpType.add)
        nc.sync.dma_start(out_ext[ds(rt * P, P), :], osb[:])


def _run(nc, in_maps):
    from concourse.bass_utils import run_bass_kernel_spmd
    res = run_bass_kernel_spmd(nc, in_maps, core_ids=list(range(8)))
    return res.results


def make_in_maps(x, adj, qkv_w, out_w, out_b):
    x = np.asarray(x, np.float32)
    adj = np.asarray(adj, np.float32)
    wt = np.ascontiguousarray(np.asarray(qkv_w, np.float32).T).astype(BF16NP)
    owt = np.ascontiguousarray(np.asarray(out_w, np.float32).T).astype(BF16NP)
    outb = np.ascontiguousarray(
        np.broadcast_to(np.asarray(out_b, np.float32), (P, C)))
    in_maps = []
    for c in range(8):
        b, half = divmod(c, 2)
        xb = np.roll(x[b], -half * NQ, axis=0)
        xt = np.ascontiguousarray(xb.T).astype(BF16NP)          # [C, N]
        aj = np.roll(adj[half * NQ:(half + 1) * NQ, :], -half * NQ, axis=1)
        ajt = np.ascontiguousarray(
            aj.T.reshape(16, P, NQ).transpose(1, 0, 2))          # [P, 16, NQ]
        in_maps.append({
            "xt": xt, "adjt": ajt,
            "qkv_wt": wt, "out_wt": owt, "out_b": outb,
        })
    return in_maps


def kernel(x, x0, adj, qkv_w, out_w, out_b, pos_w, pos_b):
    """Full-input, full-output entry point.  x0/pos_w/pos_b are dead in the
    reference computation and are ignored."""
    if "nc" not in _CACHED:
        _CACHED["nc"] = build_kernel(repeat=1)
    nc = _CACHED["nc"]
    in_maps = make_in_maps(x, adj, qkv_w, out_w, out_b)
    results = _run(nc, in_maps)
    out = np.empty((B, N, C), np.float32)
    for c in range(8):
        b, half = divmod(c, 2)
        out[b, half * NQ:(half + 1) * NQ, :] = results[c]["out"]
    return out
